# revision 49
# baseline (speedup 1.0000x reference)
"""Attention-LSTM pooling kernel for Trainium2 (8 NeuronCores, batch-parallel).

Model (per sample): emb = embedding[x]; h = LSTM(emb); a = tanh(h@W1.T+b1);
alpha = softmax(a@U over seq); ctx = sum_s alpha_s h_s; logit = ctx@W2.T+b2.

Sharding: batch 128 -> 16 samples per core, params replicated.

Per-core on-chip layout (the scan is latency-bound, so everything lives in
transposed "feature-on-partitions" form):
  - tokens ordered t-major: tok = t*16 + b_local
  - hT_all  [128, (S+1, 2, 16)] bf16   (H=256 -> 2 chunks of 128; slot 0 = h_{-1}=0)
  - xpT     [128, (t, 8, 16)] bf16 windows (4H=1024 -> 8 gate chunks of 128)
  - gates psum [128, (8 chunk, 16 b)]; chunk order i,i,f,f,o,o,g,g (host-reordered)
"""

import os
import sys

sys.path.insert(0, "/opt/trn_rl_repo")

from contextlib import ExitStack

import numpy as np
import ml_dtypes

import concourse.bass as bass
import concourse.bacc as bacc
import concourse.tile as tile
from concourse import mybir
from concourse.bass_utils import run_bass_kernel_spmd

BF16 = mybir.dt.bfloat16
F32 = mybir.dt.float32
I32 = mybir.dt.int32
AF = mybir.ActivationFunctionType
OP = mybir.AluOpType

B, S, V, E, H, A, C = 128, 512, 50000, 256, 256, 128, 2
NCORES = 8
BL = B // NCORES  # 16 samples per core
GC = 8  # gate chunks (4H/128)
KC = 2  # contraction chunks (H/128 == E/128)
EXP_SHIFT = -10.0  # exp(score - 10): guards overflow, softmax-invariant


def build_program(S_=S, V_=V, WIN=64, NHALF=2):
    """Emit the bass/tile program for one core (SPMD: all cores identical)."""
    NW = S_ // WIN
    NTOK = S_ * BL // 128  # token tiles of 128
    TPW = NTOK // NW  # token tiles per window
    nc = bacc.Bacc()  # Bacc: TRN2 sync-wait legalization + ACT table loads

    # ---- external I/O (per-core) ----
    table = nc.dram_tensor("table", [V_, E], BF16, kind="ExternalInput").ap()
    idx = nc.dram_tensor("idx", [128, NTOK], I32, kind="ExternalInput").ap()
    wih_t = nc.dram_tensor("wih_t", [E, 4 * H], BF16, kind="ExternalInput").ap()
    whh_t = nc.dram_tensor("whh_t", [H, 4 * H], BF16, kind="ExternalInput").ap()
    biasT = nc.dram_tensor("biasT", [128, GC], F32, kind="ExternalInput").ap()
    w1_t = nc.dram_tensor("w1_t", [H, A], BF16, kind="ExternalInput").ap()
    b1T = nc.dram_tensor("b1T", [A, 1], F32, kind="ExternalInput").ap()
    urep = nc.dram_tensor("urep", [A, 128], BF16, kind="ExternalInput").ap()
    w2_t = nc.dram_tensor("w2_t", [H, C], F32, kind="ExternalInput").ap()
    b2row = nc.dram_tensor("b2row", [1, C], F32, kind="ExternalInput").ap()
    ident = nc.dram_tensor("ident", [128, 128], BF16, kind="ExternalInput").ap()
    logits = nc.dram_tensor("logits", [BL, C], F32, kind="ExternalOutput").ap()

    with tile.TileContext(nc) as tc, ExitStack() as ctx:
        const = ctx.enter_context(tc.tile_pool(name="const", bufs=1))
        state = ctx.enter_context(tc.tile_pool(name="state", bufs=1))
        win = ctx.enter_context(tc.tile_pool(name="win", bufs=2))
        att = ctx.enter_context(tc.tile_pool(name="att", bufs=2))
        # one slot per token tile: indirect DMAs only support a single sync
        # wait, so gather slots must never be reused (reuse adds a PE wait)
        embp = ctx.enter_context(tc.tile_pool(name="embp", bufs=NTOK))
        work = ctx.enter_context(tc.tile_pool(name="work", bufs=3))
        pg = ctx.enter_context(tc.tile_pool(name="pg", bufs=2, space="PSUM"))
        pb = ctx.enter_context(tc.tile_pool(name="pb", bufs=2, space="PSUM"))
        # attention PSUM separate from the producer-gemm pool: sharing one
        # pool chained gemm-MM -> att-ACT -> gemm-MM into a serializing
        # dependency ring at window boundaries
        patt = ctx.enter_context(tc.tile_pool(name="patt", bufs=1, space="PSUM"))
        pfill_pool = ctx.enter_context(tc.tile_pool(name="pfill", bufs=1, space="PSUM"))
        ptr_pool = ctx.enter_context(tc.tile_pool(name="ptr", bufs=2, space="PSUM"))

        # ---- load constants ----
        def load_const(shape, dt, src, tag):
            t = const.tile(shape, dt, tag=tag)
            nc.sync.dma_start(t[:], src)
            return t

        # idx/ident first: the first window's gathers+transposes only need
        # these, so they start while the big weight DMAs stream in
        idx_sb = load_const([128, NTOK], I32, idx[:], "idx")
        id_sb = load_const([128, 128], BF16, ident[:], "ident")
        # PE warmup: ~3.2us of junk matmuls hidden under the weight DMAs, so
        # the first window's producer GEMMs run at the warm 2.4 GHz clock
        pwu = pfill_pool.tile([128, 512], F32, tag="fill")
        for _ in range(30):
            nc.tensor.matmul(
                pwu[0:128, 0:128], id_sb[:], id_sb[:],
                start=True, stop=True, skip_group_check=True,
            )
        wih_sb = [
            load_const([128, 4 * H], BF16, wih_t[k * 128 : (k + 1) * 128, :], f"wih{k}")
            for k in range(KC)
        ]
        whh_sb = [
            load_const([128, 4 * H], BF16, whh_t[k * 128 : (k + 1) * 128, :], f"whh{k}")
            for k in range(KC)
        ]
        biasT_sb = load_const([128, GC], F32, biasT[:], "biasT")
        w1_sb = [
            load_const([128, A], BF16, w1_t[k * 128 : (k + 1) * 128, :], f"w1{k}")
            for k in range(KC)
        ]
        b1_sb = load_const([A, 1], F32, b1T[:], "b1")
        urep_sb = load_const([A, 128], BF16, urep[:], "urep")
        w2_sb = [
            load_const([128, C], F32, w2_t[k * 128 : (k + 1) * 128, :], f"w2{k}")
            for k in range(KC)
        ]
        b2_sb = load_const([1, C], F32, b2row[:], "b2")
        ones_sb = const.tile([1, BL], F32, tag="ones")
        nc.vector.memset(ones_sb[:], 1.0)
        eshift_sb = const.tile([128, 1], F32, tag="eshift")
        nc.vector.memset(eshift_sb[:], EXP_SHIFT)

        # ---- persistent state ----
        # hT stores H = 2h (doubled); host pre-scales W_hh/W1/W2 by 0.5.
        hT_all = state.tile([128, (S_ + 1) * 2 * BL], BF16, tag="hT")
        hT = hT_all[:].rearrange("p (t c b) -> p t c b", t=S_ + 1, c=2, b=BL)
        nc.vector.memset(hT[:, 0], 0.0)  # h_{-1} = 0
        # chain tile: [tau_i(2BL) | tau_f(2BL) | tau_g(2BL) | c2(2BL) | tau_o(2BL)]
        # where c2 = 2c (doubled cell state); tanh(i,f,g) lands in cols
        # 0:6BL (critical path), tanh(o) in 8BL:10BL (off-path)
        chain = state.tile([128, 10 * BL], F32, tag="chain")
        nc.vector.memset(chain[:, 6 * BL : 8 * BL], 0.0)  # c2_{-1} = 0
        # attention accumulators (pooling runs per-window, inside the scan)
        ctx_acc = state.tile([128, 2 * BL], F32, tag="ctxa")
        nc.vector.memset(ctx_acc[:], 0.0)
        esum_acc = state.tile([128, BL], F32, tag="esum")
        nc.vector.memset(esum_acc[:], 0.0)

        # window schedule: small first window (shorter serial preamble) and
        # small last window (shorter attention tail)
        WINS = [32] + [64] * 7 + [32]
        NWV = len(WINS)
        STARTS = [sum(WINS[:i]) for i in range(NWV)]
        MAXW = max(WINS)
        TCH = 512 // BL  # timesteps per 512-col psum chunk

        # ---- window producer plans: gather -> transpose -> xp GEMM.
        # Emitted interleaved with the PREVIOUS window's scan steps so the
        # producer matmuls fill the scan's idle PE gaps instead of bursting
        # serially at each window boundary.
        def make_plan(wi):
            W = WINS[wi]
            embT = win.tile([128, KC * MAXW * BL], BF16, tag="embT")
            embT_v = embT[:].rearrange("p (k n) -> p k n", k=KC)
            xpT = win.tile([128, MAXW * GC * BL], BF16, tag="xpT")
            xpT_v = xpT[:].rearrange("p (t g b) -> p t g b", t=MAXW, g=GC)
            ops = [("tok", j) for j in range(W * BL // 128)]
            ops += [("gemm", j, n) for n in range(W * BL // 512) for j in range(GC)]
            return {
                "j0": STARTS[wi] * BL // 128,
                "embT_v": embT_v,
                "xpT_v": xpT_v,
                "ops": ops,
                "preamble": wi == 0,
                "gemm_i": 0,
            }

        def emit_producer_op(plan):
            if not plan["ops"]:
                return
            op = plan["ops"].pop(0)
            j0, embT_v, xpT_v = plan["j0"], plan["embT_v"], plan["xpT_v"]
            if op[0] == "tok":
                j = op[1]
                emb_sb = embp.tile([128, E], BF16, tag="emb")
                nc.gpsimd.indirect_dma_start(
                    out=emb_sb[:],
                    out_offset=None,
                    in_=table[:],
                    in_offset=bass.IndirectOffsetOnAxis(
                        ap=idx_sb[:, j0 + j : j0 + j + 1], axis=0
                    ),
                )
                for k in range(KC):
                    ptr = ptr_pool.tile([128, 128], BF16, tag="tr")
                    nc.tensor.transpose(
                        ptr[:], emb_sb[:, k * 128 : (k + 1) * 128], id_sb[:]
                    )
                    # keep producer evacs off the ACT engine (the scan
                    # chain's scarce resource); gpsimd can't read PSUM
                    nc.vector.tensor_copy(
                        embT_v[:, k, j * 128 : (j + 1) * 128], ptr[:]
                    )
            else:
                _, j, n = op
                gi = plan["gemm_i"]
                plan["gemm_i"] = gi + 1
                if plan["preamble"]:
                    # pre-scan, patt/pfill banks are idle: widen the psum
                    # rotation so the gemm->evac ring pipelines instead of
                    # ping-ponging on pb's 2 banks (this gates scan start)
                    pool, ptag = [(pb, "big"), (patt, "abig"), (pfill_pool, "fill")][
                        gi % 3
                    ]
                else:
                    pool, ptag = pb, "big"
                pxp = pool.tile([128, 512], F32, tag=ptag)
                for k in range(KC):
                    nc.tensor.matmul(
                        pxp[:],
                        wih_sb[k][:, j * 128 : (j + 1) * 128],
                        embT_v[:, k, n * 512 : (n + 1) * 512],
                        start=(k == 0),
                        stop=(k == KC - 1),
                    )
                # bias folded into the evac (per-partition bias add): saves a
                # K=1 matmul of N=512 on the PE per gemm op. Runs on ACT
                # (identity is in every ACT table set, so no table reload);
                # ACT has slack while DVE carries the chain + attention. In
                # the preamble DVE is idle too, so alternate engines there.
                out_ap = xpT_v[:, n * TCH : (n + 1) * TCH, j, :]
                in_ap = pxp[:].rearrange("p (t b) -> p t b", b=BL)
                if plan["preamble"] and gi % 2 == 1:
                    nc.vector.tensor_scalar(
                        out=out_ap, in0=in_ap,
                        scalar1=biasT_sb[:, j : j + 1], scalar2=None, op0=OP.add,
                    )
                else:
                    nc.scalar.activation(
                        out_ap, in_ap, AF.Identity, bias=biasT_sb[:, j : j + 1]
                    )

        # ---- window attention plans: aT = tanh(W1 @ h + b1); e = exp(score);
        # pooling partials. Paced into the NEXT window's scan so the big
        # ACT/DVE ops land in the chain's idle engine slots instead of
        # stalling the scan at each window boundary. ew is written in
        # (b, t) layout so the t-reductions run on contiguous elements.
        def make_att_plan(wi):
            W = WINS[wi]
            aw = att.tile([128, MAXW * BL], BF16, tag="aw")
            ew = att.tile([128, MAXW * BL], F32, tag="ew")
            ew_v = ew[:, : W * BL].rearrange("p (b t) -> p b t", b=BL)
            nch = W * BL // 512
            ops = []
            for n in range(nch):
                ops += [("aw", n), ("e", n)]
            for n in range(nch):
                ops += [("esum", n), ("esumadd", n)]
            for c in range(2):
                for n in range(nch):
                    ops += [("ctxm", c, n), ("ctxr", c, n), ("ctxa", c, n)]
            return {
                "W": W,
                "s0": STARTS[wi],
                "aw": aw,
                "ew_v": ew_v,
                "ops": ops,
                "tmp": {},
            }

        def att_psum(plan):
            # alternate between two psum banks: a single bank serializes the
            # aw-MM -> aw-ACT -> e-MM -> e-ACT ring across many scan steps
            pi = plan.setdefault("pi", 0)
            plan["pi"] = pi + 1
            pool, ptag = [(patt, "abig"), (pfill_pool, "fill")][pi % 2]
            return pool.tile([128, 512], F32, tag=ptag, name=f"apsum_{ptag}")

        def emit_att_op(plan):
            if not plan["ops"]:
                return
            op = plan["ops"].pop(0)
            W, s0, aw, ew_v = plan["W"], plan["s0"], plan["aw"], plan["ew_v"]
            tmp = plan["tmp"]
            if op[0] == "aw":
                n = op[1]
                pa = att_psum(plan)
                for k in range(KC):
                    nc.tensor.matmul(
                        pa[:],
                        w1_sb[k][:],
                        hT[:, s0 + 1 + n * TCH : s0 + 1 + (n + 1) * TCH, k, :],
                        start=(k == 0),
                        stop=(k == KC - 1),
                    )
                nc.scalar.activation(
                    aw[:, n * 512 : (n + 1) * 512], pa[:], AF.Tanh, bias=b1_sb[:]
                )
            elif op[0] == "e":
                n = op[1]
                pe = att_psum(plan)
                nc.tensor.matmul(
                    pe[:], urep_sb[:], aw[:, n * 512 : (n + 1) * 512],
                    start=True, stop=True,
                )
                nc.scalar.activation(
                    ew_v[:, :, n * TCH : (n + 1) * TCH].rearrange("p b t -> p t b"),
                    pe[:].rearrange("p (t b) -> p t b", b=BL),
                    AF.Exp,
                    bias=eshift_sb[:],
                )
            elif op[0] == "esum":
                n = op[1]
                psw = work.tile([128, BL], F32, tag="psw")
                tmp[f"psw{n}"] = psw
                nc.vector.tensor_reduce(
                    out=psw[:],
                    in_=ew_v[:, :, n * TCH : (n + 1) * TCH],
                    axis=mybir.AxisListType.X,
                    op=OP.add,
                )
            elif op[0] == "esumadd":
                n = op[1]
                nc.vector.tensor_tensor(
                    out=esum_acc[:], in0=esum_acc[:], in1=tmp[f"psw{n}"][:], op=OP.add
                )
            elif op[0] == "ctxm":
                _, c, n = op
                htld = work.tile([128, 512], F32, tag="htld")
                tmp[f"htld{c}{n}"] = htld
                nc.vector.tensor_tensor(
                    out=htld[:].rearrange("p (b t) -> p b t", b=BL),
                    in0=hT[
                        :, s0 + 1 + n * TCH : s0 + 1 + (n + 1) * TCH, c, :
                    ].rearrange("p t b -> p b t"),
                    in1=ew_v[:, :, n * TCH : (n + 1) * TCH],
                    op=OP.mult,
                )
            elif op[0] == "ctxr":
                _, c, n = op
                pcw = work.tile([128, BL], F32, tag="pcw")
                tmp[f"pcw{c}{n}"] = pcw
                nc.vector.tensor_reduce(
                    out=pcw[:],
                    in_=tmp[f"htld{c}{n}"][:].rearrange("p (b t) -> p b t", b=BL),
                    axis=mybir.AxisListType.X,
                    op=OP.add,
                )
            else:  # ctxa
                _, c, n = op
                nc.vector.tensor_tensor(
                    out=ctx_acc[:, c * BL : (c + 1) * BL],
                    in0=ctx_acc[:, c * BL : (c + 1) * BL],
                    in1=tmp[f"pcw{c}{n}"][:],
                    op=OP.add,
                )

        cur = make_plan(0)
        while cur["ops"]:
            emit_producer_op(cur)

        FILLERS = int(os.environ.get("FILLERS", "0"))
        att_prev = None  # previous window's attention plan (paced into this one)
        for wi in range(NWV):
            W = WINS[wi]
            t0 = STARTS[wi]
            xpT_v = cur["xpT_v"]
            nxt = make_plan(wi + 1) if wi + 1 < NWV else None

            # proportional pacing: spread producer ops (next window) and
            # attention ops (previous window) evenly across this window's
            # scan steps instead of front-loading them
            np_ops = len(nxt["ops"]) if nxt is not None else 0
            na_ops = len(att_prev["ops"]) if att_prev is not None else 0
            p_done = a_done = 0

            # ---- LSTM scan over this window ----
            for tl in range(W):
                t = t0 + tl
                # producers at the step top (their MMs fill the chain-phase
                # PE gap); attention at the step bottom, behind the chain in
                # the engine FIFOs
                while nxt is not None and p_done * W < (tl + 1) * np_ops:
                    emit_producer_op(nxt)
                    p_done += 1
                pgate = pg.tile([128, GC * BL], F32, tag="g")
                # seed psum with xp_t via identity matmul (keeps the
                # xp-add off the serial chain), then accumulate W_hh @ h
                nc.tensor.matmul(
                    pgate[:],
                    id_sb[:],
                    xpT_v[:, tl, :, :],
                    start=True,
                    stop=False,
                    skip_group_check=True,
                )
                # j-outer: i,f,g gate chunks (j=0..5) complete first so the
                # critical-path tanh can start before the o-chunk matmuls
                for j in range(GC):
                    for k in range(KC):
                        nc.tensor.matmul(
                            pgate[:, j * BL : (j + 1) * BL],
                            whh_sb[k][:, j * 128 : (j + 1) * 128],
                            hT[:, t, k, :],
                            start=False,
                            stop=(k == KC - 1),
                            skip_group_check=True,
                        )
                # filler matmuls: PE-warming work emitted right after the
                # burst so it runs during the chain phase, keeping the HAM
                # activity window busy (cold PE runs the whole scan at
                # 1.2 GHz otherwise). Output goes to a dead PSUM bank.
                for _ in range(FILLERS):
                    pfill = pfill_pool.tile([128, 512], F32, tag="fill")
                    nc.tensor.matmul(
                        pfill[:], id_sb[:], whh_sb[0][:, 0:512],
                        start=True, stop=True, skip_group_check=True,
                    )
                # chunk cols (xBL): i=0:2, f=2:4, o=4:6, g=6:8
                # i,f,o rows pre-scaled x0.5: sigmoid(x)=(1+tanh(x/2))/2, so
                # ONE tanh covers all four gates. State is doubled (c2=2c,
                # H=2h), absorbing the sigmoid affine into two STT ops:
                #   v = (tau[i,f]+1) * [tau_g, c2]  -> [2ig, 4fc]
                #   c2' = 0.5*v_f + v_g = 2(fc+ig)
                #   tc = tanh(0.5*c2') = tanh(c')
                #   H  = (tau_o+1)*tc = 2h   (W_hh/W1/W2 pre-scaled x0.5)
                nc.scalar.activation(
                    chain[:, 0 : 6 * BL], pgate[:, 0 : 6 * BL], AF.Tanh
                )
                v_sb = work.tile([128, 4 * BL], F32, tag="v")
                nc.vector.scalar_tensor_tensor(
                    out=v_sb[:],
                    in0=chain[:, 0 : 4 * BL],
                    scalar=1.0,
                    in1=chain[:, 4 * BL : 8 * BL],
                    op0=OP.add,
                    op1=OP.mult,
                )
                # off-path: tanh(o) on ACT while DVE runs the c2 update
                nc.scalar.activation(
                    chain[:, 8 * BL : 10 * BL], pgate[:, 6 * BL : 8 * BL], AF.Tanh
                )
                nc.vector.scalar_tensor_tensor(
                    out=chain[:, 6 * BL : 8 * BL],
                    in0=v_sb[:, 2 * BL : 4 * BL],
                    scalar=0.5,
                    in1=v_sb[:, 0 : 2 * BL],
                    op0=OP.mult,
                    op1=OP.add,
                )
                tc_sb = work.tile([128, 2 * BL], F32, tag="tc")
                nc.scalar.activation(
                    tc_sb[:], chain[:, 6 * BL : 8 * BL], AF.Tanh, scale=0.5
                )
                nc.vector.scalar_tensor_tensor(
                    out=hT_all[:, (t + 1) * 2 * BL : (t + 2) * 2 * BL],
                    in0=chain[:, 8 * BL : 10 * BL],
                    scalar=1.0,
                    in1=tc_sb[:],
                    op0=OP.add,
                    op1=OP.mult,
                )
                # attention starts at tl=8: the first window steps carry
                # boundary-crossing leftovers (late evacs, tok gathers), so
                # keep the big attention ACT/DVE ops clear of them
                while (
                    att_prev is not None
                    and tl >= 8
                    and a_done * (W - 8) < (tl - 7) * na_ops
                ):
                    emit_att_op(att_prev)
                    a_done += 1

            if nxt is not None:
                while nxt["ops"]:  # flush any unpaced producer ops
                    emit_producer_op(nxt)
            if att_prev is not None:
                while att_prev["ops"]:  # flush any unpaced attention ops
                    emit_att_op(att_prev)
            att_prev = make_att_plan(wi)
            cur = nxt

        # tail: attention of the final window
        while att_prev["ops"]:
            emit_att_op(att_prev)

        # ---- finish: ctx = ctx_acc / esum ; logits = ctx @ W2.T + b2 ----
        rsum = work.tile([128, BL], F32, tag="rsum")
        nc.vector.reciprocal(rsum[:], esum_acc[:])
        ctxn = work.tile([128, 2 * BL], F32, tag="ctxn")
        for cch in range(2):
            nc.vector.tensor_tensor(
                out=ctxn[:, cch * BL : (cch + 1) * BL],
                in0=ctx_acc[:, cch * BL : (cch + 1) * BL],
                in1=rsum[:],
                op=OP.mult,
            )
        # reuse the filler PSUM bank for the tiny final-logits accumulation
        plog_t = pfill_pool.tile([128, 512], F32, tag="fill")
        plog = plog_t[0:BL, 0:C]
        for cch in range(2):
            nc.tensor.matmul(
                plog, ctxn[:, cch * BL : (cch + 1) * BL], w2_sb[cch][:],
                start=(cch == 0), stop=False,
            )
        nc.tensor.matmul(plog, ones_sb[:], b2_sb[:], start=False, stop=True)
        out_sb = work.tile([BL, C], F32, tag="outsb")
        nc.vector.tensor_copy(out_sb[:], plog)
        nc.sync.dma_start(logits[:], out_sb[:])

    nc.finalize()
    return nc


def prep_inputs(x, embedding, W_ih, W_hh, b_ih, b_hh, W1, b1, U, W2, b2, S_=S, V_=V):
    """Host-side parameter prep + per-core input maps."""
    bf = ml_dtypes.bfloat16
    # gates stay in torch order [i,f,g,o]; tanh(i,f,g) is the critical-path
    # ACT, tanh(o) runs off-path
    perm = np.arange(4 * H)
    # i,f,o pre-scaled x0.5: sigmoid(x) = (1 + tanh(x/2))/2, so the whole
    # kernel needs only the {tanh, exp} ACT table set (no mid-scan reloads)
    gsc = np.ones((4 * H, 1), np.float32)
    gsc[: 2 * H] = 0.5
    gsc[3 * H :] = 0.5
    wih_r = np.asarray(W_ih)[perm] * gsc  # [4H, E]
    # extra x0.5 on the h-input side: hT stores H = 2h
    whh_r = np.asarray(W_hh)[perm] * gsc * 0.5
    bias_r = (np.asarray(b_ih) + np.asarray(b_hh))[perm] * gsc[:, 0]  # [4H]

    common = {
        "table": np.ascontiguousarray(np.asarray(embedding).astype(bf)),
        "wih_t": np.ascontiguousarray(wih_r.T.astype(bf)),  # [E, 4H]
        "whh_t": np.ascontiguousarray(whh_r.T.astype(bf)),  # [H, 4H]
        "biasT": np.ascontiguousarray(
            bias_r.reshape(GC, 128).T.astype(np.float32)
        ),
        "w1_t": np.ascontiguousarray((np.asarray(W1).T * 0.5).astype(bf)),  # [H, A]
        "b1T": np.ascontiguousarray(np.asarray(b1).reshape(A, 1).astype(np.float32)),
        "urep": np.ascontiguousarray(
            np.repeat(np.asarray(U).astype(np.float32), 128, axis=1).astype(bf)
        ),
        "w2_t": np.ascontiguousarray(
            (np.asarray(W2).T * 0.5).astype(np.float32)
        ),  # [H, C]
        "b2row": np.ascontiguousarray(np.asarray(b2).reshape(1, C).astype(np.float32)),
        "ident": np.eye(128, dtype=np.float32).astype(bf),
    }
    x = np.asarray(x)
    in_maps = []
    for c in range(NCORES):
        xs = x[c * BL : (c + 1) * BL]  # [BL, S]
        # token order t-major: tok = t*BL + b ; tile j rows p -> tok = j*128+p
        toks = xs.T.reshape(-1)  # [S*BL]
        ntok = S_ * BL // 128
        idx_np = toks.reshape(ntok, 128).T.copy().astype(np.int32)  # [128, NTOK]
        in_maps.append({**common, "idx": idx_np})
    return in_maps


_prog_cache = {}


def kernel(x, embedding, W_ih, W_hh, b_ih, b_hh, W1, b1, U, W2, b2):
    key = "full"
    if key not in _prog_cache:
        import os
        _prog_cache[key] = build_program(NHALF=int(os.environ.get('NHALF', '1')))
    nc = _prog_cache[key]
    in_maps = prep_inputs(x, embedding, W_ih, W_hh, b_ih, b_hh, W1, b1, U, W2, b2)
    res = run_bass_kernel_spmd(nc, in_maps, list(range(NCORES)))
    kernel.last_results = res  # exec_time_ns/profile when BASS_TRACE=1
    out = np.concatenate([res.results[c]["logits"] for c in range(NCORES)], axis=0)
    return out.astype(np.float32)


if __name__ == "__main__":
    import reference

    inputs = {k: np.asarray(v) for k, v in reference.setup_inputs().items()}
    got = kernel(**inputs)
    exp = np.asarray(reference.reference(**inputs))
    rel = np.abs(got - exp).max() / np.abs(exp).max()
    print("Relative error:", rel)

